# revision 1
# baseline (speedup 1.0000x reference)
"""ColorHistogramLoss Trainium2 kernel.

Math: reference soft-histogram weight for pixel x and bin k is
    w = exp(-(x - c_k)^2 / (2 sigma^2)),  sigma = bin_width = 1/64, c_k = (k+0.5)/64
In bin units u = 64x:  w = exp(-(u - (k+0.5))^2 / 2).
With y = x - 0.5 (exact in fp32) and e_k = (k+0.5) - 32:
    t = 64y - e_k,   t^2/2 = 2048 y^2 - 64 e_k y + e_k^2/2
So per (pixel, bin):
    w = Exp( -(2048 y^2 - 64 e_k y) - e_k^2/2 )
The quadratic form rides the TensorEngine as a K=4 constant-stationary matmul
(rows = [yA, yA^2, yB, yB^2] for two images packed on 128 PSUM partitions =
2 x 64 bins), then a single ScalarEngine Exp pass with per-partition bias
-e_k^2/2 and fused accum_out produces per-chunk bin sums.  Host folds the
per-chunk partials in fp64, cumsums, normalizes, and takes the L1 mean.

Sharding: each of the 8 cores processes a 1/8 pixel-slice of all 24 images
(12 pred + 12 target); partial histogram sums are combined on host.
"""

import os

import numpy as np

N_CORES = 8
B, C, H, W = 4, 3, 256, 256
NIMG = 2 * B * C          # 24 images (12 pred + 12 target)
NPX = H * W               # 65536 pixels / image
PXC = NPX // N_CORES      # 8192 pixels / image / core
NPAIR = NIMG // 2         # 12 image pairs packed per matmul column-block
CHUNK = 512               # pixels per matmul (f32 moving-operand max)
ACHUNK = 1024             # pixels per ACT op (2 PSUM banks)
NCH = PXC // ACHUNK       # 8 ACT chunks per pair per core
NCOL = NPAIR * NCH        # 96 accumulator columns
BINS = 64
WIDE_F = NIMG * PXC // 128  # 1536 free-dim of the wide prep layout

_CACHE = {}


def _consts():
    k = np.arange(128) % 64
    c = k + 0.5
    e = c - 32.0
    stat = np.zeros((NPAIR, 2 * NIMG, 128), np.float32)
    for j in range(NPAIR):
        stat[j, 4 * j + 0, :64] = -64.0 * e[:64]
        stat[j, 4 * j + 1, :64] = 2048.0
        stat[j, 4 * j + 2, 64:] = -64.0 * e[64:]
        stat[j, 4 * j + 3, 64:] = 2048.0
    biasd = (-(e * e) / 2.0).astype(np.float32).reshape(128, 1)
    return stat, biasd


def _build():
    import concourse.bacc as bacc
    import concourse.tile as tile
    import concourse.mybir as mybir

    f32 = mybir.dt.float32
    nc = bacc.Bacc("TRN2", target_bir_lowering=False, debug=False,
                   num_devices=N_CORES)

    xin = nc.dram_tensor("xin", [NIMG, PXC], f32, kind="ExternalInput")
    stat = nc.dram_tensor("stat", [NPAIR, 2 * NIMG, 128], f32,
                          kind="ExternalInput")
    biasd = nc.dram_tensor("biasd", [128, 1], f32, kind="ExternalInput")
    prep = nc.dram_tensor("prep", [2 * NIMG, PXC], f32)
    out = nc.dram_tensor("acc_out", [128, NCOL], f32, kind="ExternalOutput")

    with tile.TileContext(nc) as tc:
        with (
            tc.tile_pool(name="p_const", bufs=1) as cpool,
            tc.tile_pool(name="p_wide", bufs=1) as wpool,
            tc.tile_pool(name="p_pair", bufs=1) as ppool,
            tc.tile_pool(name="p_scr", bufs=2) as spool,
            tc.tile_pool(name="p_acc", bufs=1) as apool,
            tc.tile_pool(name="p_psum", bufs=3, space="PSUM") as qpool,
        ):
            stat_t = cpool.tile([2 * NIMG, NPAIR * 128], f32)
            nc.sync.dma_start(
                out=stat_t[:].rearrange("k (j m) -> k j m", m=128),
                in_=stat[:].rearrange("j k m -> k j m"),
            )
            bias_t = cpool.tile([128, 1], f32)
            nc.sync.dma_start(out=bias_t[:], in_=biasd[:])

            # wide layout: partition p, col i*64+c  <=  xin[i, p*64+c]
            xw = wpool.tile([128, WIDE_F], f32)
            nc.sync.dma_start(
                out=xw[:].rearrange("p (i c) -> p i c", c=PXC // 128),
                in_=xin[:].rearrange("i (p c) -> p i c", p=128),
            )
            yw = wpool.tile([128, WIDE_F], f32)
            nc.vector.tensor_scalar_add(out=yw[:], in0=xw[:], scalar1=-0.5)
            ysq = wpool.tile([128, WIDE_F], f32)
            nc.vector.tensor_mul(out=ysq[:], in0=yw[:], in1=yw[:])

            # prep rows 2i = y_i, 2i+1 = y_i^2
            prep_v = prep[:].rearrange("(i two) (p c) -> two p i c",
                                       two=2, p=128)
            nc.sync.dma_start(
                out=prep_v[0],
                in_=yw[:].rearrange("p (i c) -> p i c", c=PXC // 128),
            )
            nc.sync.dma_start(
                out=prep_v[1],
                in_=ysq[:].rearrange("p (i c) -> p i c", c=PXC // 128),
            )

            acc = apool.tile([128, NCOL], f32)
            # whole prep resident: [48 partitions, 8192] = 32KB/partition
            pt = ppool.tile([2 * NIMG, PXC], f32)
            nc.sync.dma_start(out=pt[:], in_=prep[:])
            for j in range(NPAIR):
                for ch in range(NCH):
                    ps = qpool.tile([128, ACHUNK], f32, tag="ps")
                    for h in range(ACHUNK // CHUNK):
                        px0 = ACHUNK * ch + CHUNK * h
                        nc.tensor.matmul(
                            out=ps[:, CHUNK * h:CHUNK * (h + 1)],
                            lhsT=stat_t[:, 128 * j:128 * (j + 1)],
                            rhs=pt[:, px0:px0 + CHUNK],
                            start=True, stop=True,
                        )
                    scr = spool.tile([128, ACHUNK], f32, tag="scr")
                    col = NCH * j + ch
                    nc.scalar.activation(
                        out=scr[:], in_=ps[:],
                        func=mybir.ActivationFunctionType.Exp,
                        bias=bias_t[:, 0:1], scale=-1.0,
                        accum_out=acc[:, col:col + 1],
                    )
            nc.sync.dma_start(out=out[:], in_=acc[:])
    if not nc.is_finalized():
        nc.finalize()
    return nc


def kernel(pred: np.ndarray, target: np.ndarray) -> np.ndarray:
    X = np.concatenate(
        [np.asarray(pred, np.float32).reshape(B * C, NPX),
         np.asarray(target, np.float32).reshape(B * C, NPX)], axis=0)

    if "nc" not in _CACHE:
        _CACHE["nc"] = _build()
    nc = _CACHE["nc"]

    statM, biasv = _consts()
    in_maps = [
        {"xin": np.ascontiguousarray(X[:, c * PXC:(c + 1) * PXC]),
         "stat": statM, "biasd": biasv}
        for c in range(N_CORES)
    ]

    from concourse.bass_utils import run_bass_kernel_spmd
    trace = bool(int(os.environ.get("KERNEL_TRACE", "0")))
    res = run_bass_kernel_spmd(nc, in_maps, core_ids=list(range(N_CORES)),
                               trace=trace)
    if res.exec_time_ns:
        _CACHE["exec_time_ns"] = res.exec_time_ns

    A = np.stack([r["acc_out"] for r in res.results]).astype(np.float64)
    # [cores, 128, NCOL] -> per (partition, pair) sums
    M = A.reshape(N_CORES, 128, NPAIR, NCH).sum(axis=(0, 3))  # [128, 12]
    Hh = np.empty((NIMG, BINS), np.float64)
    for j in range(NPAIR):
        Hh[2 * j] = M[:64, j]
        Hh[2 * j + 1] = M[64:, j]
    cum = np.cumsum(Hh, axis=1)
    den = cum[:, -1:] + 1e-8
    cdf = cum / den
    loss = np.mean(np.abs(cdf[:B * C] - cdf[B * C:]))
    return np.array(loss, dtype=np.float32)



# revision 2
# speedup vs baseline: 3.0117x; 3.0117x over previous
"""ColorHistogramLoss Trainium2 kernel.

Math: reference soft-histogram weight for pixel x and bin k is
    w = exp(-(x - c_k)^2 / (2 sigma^2)),  sigma = bin_width = 1/64, c_k = (k+0.5)/64
In bin units u = 64x:  w = exp(-(u - (k+0.5))^2 / 2) = exp(-t^2/2),
t = 64x - (k+0.5).

t is LINEAR in x, so the TensorEngine computes it as a K=25 fp32r
matmul (24 image rows + a ones row; stationary coefficients 64 and
-(k+0.5), both exact in fp32r).  fp32r streams 1 column/cycle (vs 4
for fp32).  A single ScalarEngine pass with func=Derivative_Erf
(d/dx erf = 2/sqrt(pi) exp(-x^2)) and scale=1/sqrt(2) then yields
w' = (2/sqrt(pi)) exp(-t^2/2) -- the constant factor cancels in the
histogram normalization -- with fused accum_out producing per-bin
partial sums.  The HW Derivative_Erf table returns ~0 (<3e-11) for
|t| > 7, so far bins contribute nothing.

128 PSUM partitions = 2 images x 64 bins (pred_j on 0..63, target_j
on 64..127), so each of the 12 (pred, target) pairs streams the
8192-pixel slice once: 98304 PE columns and 98304 ACT elements/lane
per core.  The ACT engine (1 elem/cycle/lane @ 1.2 GHz) is the
bottleneck at ~82 us; PE (~41 us) and DMA (~13 us) hide under it.

Sharding: each of the 8 cores processes a 1/8 pixel-slice of all 24
images; per-chunk bin sums are combined on host (fp64 fold, cumsum,
normalize, L1 mean).
"""

import os

import numpy as np

N_CORES = 8
B, C, H, W = 4, 3, 256, 256
NIMG = 2 * B * C          # 24 images (12 pred + 12 target)
NPX = H * W               # 65536 pixels / image
PXC = NPX // N_CORES      # 8192 pixels / image / core
NPAIR = NIMG // 2         # 12 (pred, target) pairs
CHUNK = 512               # pixels per matmul (PSUM bank = 512 f32)
ACHUNK = 2048             # pixels per ACT op (4 PSUM banks)
NCH = PXC // ACHUNK       # 4 ACT chunks per pair per core
NCOL = NPAIR * NCH        # 48 accumulator columns
BINS = 64
KROWS = NIMG + 1          # 24 image rows + ones row

_CACHE = {}


def _consts():
    k = np.arange(128) % 64
    stat = np.zeros((KROWS, NPAIR * 128), np.float32)
    for j in range(NPAIR):
        cols = slice(128 * j, 128 * (j + 1))
        s = np.zeros((KROWS, 128), np.float32)
        s[j, :64] = 64.0          # pred_j -> bins 0..63
        s[NPAIR + j, 64:] = 64.0  # target_j -> bins 64..127
        s[NIMG] = -(k + 0.5)      # ones row
        stat[:, cols] = s
    return stat


def _build():
    import concourse.bacc as bacc
    import concourse.tile as tile
    import concourse.mybir as mybir

    f32 = mybir.dt.float32
    f32r = mybir.dt.float32r
    nc = bacc.Bacc("TRN2", target_bir_lowering=False, debug=False,
                   num_devices=N_CORES)

    xin = nc.dram_tensor("xin", [KROWS, PXC], f32r, kind="ExternalInput")
    stat = nc.dram_tensor("stat", [KROWS, NPAIR * 128], f32r,
                          kind="ExternalInput")
    out = nc.dram_tensor("acc_out", [128, NCOL], f32, kind="ExternalOutput")

    inv_sqrt2 = float(1.0 / np.sqrt(2.0))

    with tile.TileContext(nc) as tc:
        with (
            tc.tile_pool(name="p_const", bufs=1) as cpool,
            tc.tile_pool(name="p_x", bufs=1) as xpool,
            tc.tile_pool(name="p_scr", bufs=2) as spool,
            tc.tile_pool(name="p_acc", bufs=1) as apool,
            tc.tile_pool(name="p_psum", bufs=2, space="PSUM") as qpool,
        ):
            stat_t = cpool.tile([KROWS, NPAIR * 128], f32r)
            nc.sync.dma_start(out=stat_t[:], in_=stat[:])

            xts = []
            for ch in range(NCH):
                xt = xpool.tile([KROWS, ACHUNK], f32r, tag=f"x{ch}")
                nc.sync.dma_start(
                    out=xt[:], in_=xin[:, ACHUNK * ch:ACHUNK * (ch + 1)])
                xts.append(xt)

            acc = apool.tile([128, NCOL], f32)
            for ch in range(NCH):
                for j in range(NPAIR):
                    ps = qpool.tile([128, ACHUNK], f32, tag="ps")
                    for h in range(ACHUNK // CHUNK):
                        nc.tensor.matmul(
                            out=ps[:, CHUNK * h:CHUNK * (h + 1)],
                            lhsT=stat_t[:, 128 * j:128 * (j + 1)],
                            rhs=xts[ch][:, CHUNK * h:CHUNK * (h + 1)],
                            start=True, stop=True,
                        )
                    scr = spool.tile([128, ACHUNK], f32, tag="scr")
                    col = NPAIR * ch + j
                    nc.scalar.activation(
                        out=scr[:], in_=ps[:],
                        func=mybir.ActivationFunctionType.Derivative_Erf,
                        scale=inv_sqrt2,
                        accum_out=acc[:, col:col + 1],
                    )
            nc.sync.dma_start(out=out[:], in_=acc[:])
    if not nc.is_finalized():
        nc.finalize()
    return nc


def kernel(pred: np.ndarray, target: np.ndarray) -> np.ndarray:
    X = np.concatenate(
        [np.asarray(pred, np.float32).reshape(B * C, NPX),
         np.asarray(target, np.float32).reshape(B * C, NPX)], axis=0)

    if "nc" not in _CACHE:
        _CACHE["nc"] = _build()
    nc = _CACHE["nc"]

    statM = _consts()
    in_maps = []
    for c in range(N_CORES):
        xc = np.empty((KROWS, PXC), np.float32)
        xc[:NIMG] = X[:, c * PXC:(c + 1) * PXC]
        xc[NIMG] = 1.0
        in_maps.append({"xin": xc, "stat": statM})

    from concourse.bass_utils import run_bass_kernel_spmd
    trace = bool(int(os.environ.get("KERNEL_TRACE", "0")))
    res = run_bass_kernel_spmd(nc, in_maps, core_ids=list(range(N_CORES)),
                               trace=trace)
    if res.exec_time_ns:
        _CACHE["exec_time_ns"] = res.exec_time_ns

    A = np.stack([r["acc_out"] for r in res.results]).astype(np.float64)
    # [cores, 128, NCOL] -> per (partition, pair) sums
    M = A.reshape(N_CORES, 128, NCH, NPAIR).sum(axis=(0, 2))  # [128, 12]
    Hh = np.empty((NIMG, BINS), np.float64)
    for j in range(NPAIR):
        Hh[2 * j] = M[:64, j]
        Hh[2 * j + 1] = M[64:, j]
    cum = np.cumsum(Hh, axis=1)
    den = cum[:, -1:] + 1e-8
    cdf = cum / den
    loss = np.mean(np.abs(cdf[:: 2] - cdf[1:: 2]))
    return np.array(loss, dtype=np.float32)
